# revision 14
# baseline (speedup 1.0000x reference)
"""Bahdanau-style additive attention on 8 TRN2 NeuronCores.

Data-parallel over batch (32 -> 4 per core); zero collectives (softmax is
over the sequence axis, which stays local to each core).

Per-core pipeline (S=2048 local seq, B=4 local batch, E=H=1024):
  - cast enc f32->bf16 into DRAM scratch chunks (SWDGE cast-DMA)
  - DMA-transpose bf16 chunks into [e, r] tiles (xbar transpose)
  - proj_enc: PE matmul, stationary w_encT (bf16), out psum [h=128, r=512]
  - tanh(+proj_dec bias) on ACT -> bf16 activations
  - logits: PE matmul with w_out (M=1), accumulated over h-tiles
  - mask + softmax per batch column on DVE/ACT (free-dim reductions)
  - attn_response: PE matmul, stationary = weights column, moving = enc bf16
"""

import sys

import numpy as np

_BASS_PATH = "/opt/trn_rl_repo"
if _BASS_PATH not in sys.path:
    sys.path.insert(0, _BASS_PATH)

S, B_FULL, E, H = 2048, 32, 1024, 1024
NCORES = 8
B = B_FULL // NCORES            # 4 batch columns per core
NEG_BIG = 2.0e9
SCH = 512                       # seq rows per chunk
NCHUNK_B = S // SCH             # 4 chunks per batch column
ET, HT, ST = E // 128, H // 128, S // 128

_CACHE = {}


def _build():
    import concourse.tile as tile
    from concourse import bacc, mybir
    from concourse.masks import make_identity
    from contextlib import ExitStack

    F32 = mybir.dt.float32
    BF16 = mybir.dt.bfloat16
    U8 = mybir.dt.uint8
    AF = mybir.ActivationFunctionType
    ALU = mybir.AluOpType
    AX = mybir.AxisListType

    nc = bacc.Bacc("TRN2", target_bir_lowering=False, debug=False, num_devices=1)

    enc_ext = nc.declare_dram_parameter("enc", [S, B, E], F32, isOutput=False)
    mask_ext = nc.declare_dram_parameter("mask", [S, B], U8, isOutput=False)
    dec_ext = nc.declare_dram_parameter("dec", [B, E], F32, isOutput=False)
    wenc_ext = nc.declare_dram_parameter("w_enc", [H, E], F32, isOutput=False)
    wdec_ext = nc.declare_dram_parameter("w_dec", [H, E], F32, isOutput=False)
    wout_ext = nc.declare_dram_parameter("w_out", [1, H], F32, isOutput=False)
    outw_ext = nc.declare_dram_parameter("out_w", [S, B], F32, isOutput=True)
    outr_ext = nc.declare_dram_parameter("out_resp", [B, E], F32, isOutput=True)

    with tile.TileContext(nc) as tc, ExitStack() as ctx:
        dram1 = ctx.enter_context(tc.tile_pool(name="dram1", bufs=1, space="DRAM"))
        dramc = ctx.enter_context(tc.tile_pool(name="dramc", bufs=16, space="DRAM"))
        persist = ctx.enter_context(tc.tile_pool(name="persist", bufs=1))
        sb_encT = ctx.enter_context(tc.tile_pool(name="sb_encT", bufs=24))
        sb_l = ctx.enter_context(tc.tile_pool(name="sb_l", bufs=2))
        sb_act = ctx.enter_context(tc.tile_pool(name="sb_act", bufs=10))
        sb_resp = ctx.enter_context(tc.tile_pool(name="sb_resp", bufs=6))
        sb_misc = ctx.enter_context(tc.tile_pool(name="sb_misc", bufs=1))
        ps_mm = ctx.enter_context(tc.tile_pool(name="ps_mm", bufs=2, space="PSUM"))
        ps_l = ctx.enter_context(tc.tile_pool(name="ps_l", bufs=2, space="PSUM"))
        ps_r = ctx.enter_context(tc.tile_pool(name="ps_r", bufs=2, space="PSUM"))
        ps_t = ctx.enter_context(tc.tile_pool(name="ps_t", bufs=2, space="PSUM"))

        # ---- phase 0: weight prep ----
        wenc_bf = dram1.tile([H, E], BF16, tag="wencbf")
        wdec_bf = dram1.tile([H, E], BF16, tag="wdecbf")
        nc.gpsimd.dma_start(out=wenc_bf[:, :], in_=wenc_ext[:, :])
        nc.gpsimd.dma_start(out=wdec_bf[:, :], in_=wdec_ext[:, :])
        wencT = persist.tile([128, ET, H], BF16, tag="wencT")
        wdecT = persist.tile([128, ET, H], BF16, tag="wdecT")
        for et in range(ET):
            nc.sync.dma_start_transpose(out=wencT[:, et, :], in_=wenc_bf[:, et * 128:(et + 1) * 128])
            nc.sync.dma_start_transpose(out=wdecT[:, et, :], in_=wdec_bf[:, et * 128:(et + 1) * 128])

        ident = persist.tile([128, 128], F32, tag="ident")
        make_identity(nc, ident[:, :])

        dec_sb = persist.tile([B, E], F32, tag="dec_sb")
        nc.sync.dma_start(out=dec_sb[:, :], in_=dec_ext[:, :])
        decT = persist.tile([128, ET, B], BF16, tag="decT")
        for dt_ in range(ET):
            tp = ps_t.tile([128, B], F32, tag="tp")
            nc.tensor.transpose(tp[:, :], dec_sb[:, dt_ * 128:(dt_ + 1) * 128], ident[0:B, 0:B])
            nc.vector.tensor_copy(decT[:, dt_, :], tp[:, :])

        wout_sb = persist.tile([1, H], F32, tag="wout_sb")
        nc.sync.dma_start(out=wout_sb[:, :], in_=wout_ext[:, :])
        woutT = persist.tile([128, HT, 1], BF16, tag="woutT")
        for ht in range(HT):
            tp = ps_t.tile([128, 1], F32, tag="tp")
            nc.tensor.transpose(tp[:, :], wout_sb[0:1, ht * 128:(ht + 1) * 128], ident[0:1, 0:1])
            nc.vector.tensor_copy(woutT[:, ht, :], tp[:, :])

        # proj_dec[h, b] in f32 (tanh bias)
        pd_sb = persist.tile([128, HT, B], F32, tag="pd_sb")
        for ht in range(HT):
            pp = ps_t.tile([128, B], F32, tag="tp")
            for dt_ in range(ET):
                nc.tensor.matmul(pp[:, :], wdecT[:, dt_, ht * 128:(ht + 1) * 128], decT[:, dt_, :],
                                 start=(dt_ == 0), stop=(dt_ == ET - 1))
            nc.vector.tensor_copy(pd_sb[:, ht, :], pp[:, :])

        # mask stays interleaved [s, b] on partition 0; rows extracted on the fly
        mask_sb = persist.tile([1, S, B], U8, tag="mask_sb")
        nc.sync.dma_start(out=mask_sb[:, :, :], in_=mask_ext[:, :])

        # logits / weights live interleaved in the [s, b] output staging row;
        # all softmax ops run in-place on stride-B views of it
        wflat = persist.tile([1, S, B], F32, tag="wflat")
        wTbf = persist.tile([128, ST, B], BF16, tag="wTbf")
        resp_sb = persist.tile([1, B, E], F32, tag="resp_sb")

        enc_bf_tiles = []

        def emit_softmax_and_resp(b, lrow):
            lb = lrow[:, :]
            mrow = sb_misc.tile([1, S], F32, tag="mrow")
            nc.vector.tensor_copy(mrow[:, :], mask_sb[:, :, b])
            # masked = logits*m + (m*2e9 - 2e9)
            nc.vector.tensor_mul(lb, lb, mrow[:, :])
            nc.vector.tensor_scalar(mrow[:, :], mrow[:, :], NEG_BIG, -NEG_BIG,
                                    ALU.mult, ALU.add)
            nc.vector.tensor_add(lb, lb, mrow[:, :])
            nmax = sb_misc.tile([1, 1], F32, tag="nmax")
            nc.vector.tensor_reduce(nmax[:, :], lb, AX.X, ALU.max, negate=True)
            ssum = sb_misc.tile([1, 1], F32, tag="ssum")
            nc.scalar.activation(lb, lb, AF.Exp, bias=nmax[:, :], scale=1.0,
                                 accum_out=ssum[:, :])
            rinv = sb_misc.tile([1, 1], F32, tag="rinv")
            nc.vector.reciprocal(rinv[:, :], ssum[:, :])
            nc.vector.tensor_scalar_mul(lb, lb, rinv[:, :])
            # stage the [s, b]-interleaved output row + transpose for the resp matmul
            nc.vector.tensor_copy(wflat[:, :, b], lb)
            for st in range(ST):
                tp = ps_t.tile([128, 1], F32, tag="tp")
                nc.tensor.transpose(tp[:, :], lrow[:, st * 128:(st + 1) * 128],
                                    ident[0:1, 0:1])
                nc.vector.tensor_copy(wTbf[:, st, b:b + 1], tp[:, :])
            # attn_response[b, :] = sum_s weights[s] * enc[s, b, :]
            psR = [ps_r.tile([1, 512], F32, tag="psR", name=f"psR{b}_{eh}")
                   for eh in range(2)]
            for st in range(ST):
                rhs = sb_resp.tile([128, E], BF16, tag="rrhs")
                cidx = b * NCHUNK_B + st // 4
                srow = (st % 4) * 128
                nc.gpsimd.dma_start(out=rhs[:, :], in_=enc_bf_tiles[cidx][srow:srow + 128, :])
                for eh in range(2):
                    nc.tensor.matmul(psR[eh][:, :], wTbf[:, st, b:b + 1],
                                     rhs[:, eh * 512:(eh + 1) * 512],
                                     start=(st == 0), stop=(st == ST - 1))
            for eh in range(2):
                nc.vector.tensor_copy(resp_sb[:, b, eh * 512:(eh + 1) * 512], psR[eh][:, :])
            nc.sync.dma_start(out=outr_ext[b:b + 1, :], in_=resp_sb[:, b, :])

        # ---- main loop over (b, s-chunk) ----
        lrow = None
        for c in range(B * NCHUNK_B):
            b, sc = divmod(c, NCHUNK_B)
            s0 = sc * SCH
            if sc == 0:
                lrow = sb_l.tile([1, S], F32, tag="lrow", name=f"lrow{b}")
            ebf = dramc.tile([SCH, E], BF16, tag="encbf")
            enc_bf_tiles.append(ebf)
            nc.gpsimd.dma_start(out=ebf[:, :], in_=enc_ext[s0:s0 + SCH, b, :])
            encT = []
            for et in range(ET):
                t = sb_encT.tile([128, SCH], BF16, tag="encT")
                nc.sync.dma_start_transpose(out=t[:, :], in_=ebf[:, et * 128:(et + 1) * 128])
                encT.append(t)
            lps = ps_l.tile([1, SCH], F32, tag="lps")
            pend = []
            for ht in range(HT):
                ps = ps_mm.tile([128, SCH], F32, tag="psmm")
                for et in range(ET):
                    nc.tensor.matmul(ps[:, :], wencT[:, et, ht * 128:(ht + 1) * 128], encT[et][:, :],
                                     start=(et == 0), stop=(et == ET - 1))
                act = sb_act.tile([128, SCH], BF16, tag="act")
                nc.scalar.activation(act[:, :], ps[:, :], AF.Tanh, bias=pd_sb[:, ht, b:b + 1],
                                     scale=1.0)
                pend.append((ht, act))
                # emit logits matmuls one h-tile behind so the PE never waits on ACT
                if len(pend) >= 2:
                    h0, a0 = pend.pop(0)
                    nc.tensor.matmul(lps[:, :], woutT[:, h0, :], a0[:, :],
                                     start=(h0 == 0), stop=(h0 == HT - 1))
            for h0, a0 in pend:
                nc.tensor.matmul(lps[:, :], woutT[:, h0, :], a0[:, :],
                                 start=(h0 == 0), stop=(h0 == HT - 1))
            nc.vector.tensor_copy(lrow[:, s0:s0 + SCH], lps[:, :])
            if sc == NCHUNK_B - 1:
                emit_softmax_and_resp(b, lrow)

        nc.sync.dma_start(out=outw_ext[:, :], in_=wflat[:, :, :])

    nc.compile()
    return nc


def _get_nc():
    if "nc" not in _CACHE:
        _CACHE["nc"] = _build()
    return _CACHE["nc"]


def kernel(enc_states, enc_mask, dec_state, w_enc, w_dec, w_out):
    from concourse.bass_utils import run_bass_kernel_spmd

    nc = _get_nc()
    enc_states = np.asarray(enc_states, dtype=np.float32)
    mask_u8 = np.asarray(enc_mask).astype(np.uint8)
    dec_state = np.asarray(dec_state, dtype=np.float32)
    w_enc = np.ascontiguousarray(np.asarray(w_enc, dtype=np.float32))
    w_dec = np.ascontiguousarray(np.asarray(w_dec, dtype=np.float32))
    w_out = np.ascontiguousarray(np.asarray(w_out, dtype=np.float32).reshape(1, H))

    in_maps = []
    for c in range(NCORES):
        bs = slice(c * B, (c + 1) * B)
        in_maps.append({
            "enc": np.ascontiguousarray(enc_states[:, bs, :]),
            "mask": np.ascontiguousarray(mask_u8[:, bs]),
            "dec": np.ascontiguousarray(dec_state[bs, :]),
            "w_enc": w_enc,
            "w_dec": w_dec,
            "w_out": w_out,
        })

    res = run_bass_kernel_spmd(nc, in_maps, core_ids=list(range(NCORES)))
    attn_weights = np.concatenate([res.results[c]["out_w"] for c in range(NCORES)], axis=1)
    attn_response = np.concatenate([res.results[c]["out_resp"] for c in range(NCORES)], axis=0)
    return attn_response.astype(np.float32), attn_weights.astype(np.float32)


# revision 23
# speedup vs baseline: 1.0127x; 1.0127x over previous
"""Bahdanau-style additive attention on 8 TRN2 NeuronCores.

Data-parallel over batch (32 -> 4 per core); zero collectives (softmax is
over the sequence axis, which stays local to each core).

Per-core pipeline (S=2048 local seq, B=4 local batch, E=H=1024):
  - cast enc f32->bf16 into DRAM scratch chunks (SWDGE cast-DMA)
  - DMA-transpose bf16 chunks into [e, r] tiles (xbar transpose)
  - proj_enc: PE matmul, stationary w_encT (bf16), out psum [h=128, r=512]
  - tanh(+proj_dec bias) on ACT -> bf16 activations
  - logits: PE matmul with w_out (M=1), accumulated over h-tiles
  - mask + softmax per batch column on DVE/ACT (free-dim reductions)
  - attn_response: PE matmul, stationary = weights column, moving = enc bf16
"""

import sys

import numpy as np

_BASS_PATH = "/opt/trn_rl_repo"
if _BASS_PATH not in sys.path:
    sys.path.insert(0, _BASS_PATH)

S, B_FULL, E, H = 2048, 32, 1024, 1024
NCORES = 8
B = B_FULL // NCORES            # 4 batch columns per core
NEG_BIG = 2.0e9
SCH = 512                       # seq rows per chunk
NCHUNK_B = S // SCH             # 4 chunks per batch column
ET, HT, ST = E // 128, H // 128, S // 128

_CACHE = {}


def _build():
    import concourse.tile as tile
    from concourse import bacc, mybir
    from concourse.masks import make_identity
    from contextlib import ExitStack

    F32 = mybir.dt.float32
    BF16 = mybir.dt.bfloat16
    U8 = mybir.dt.uint8
    F32R = mybir.dt.float32r
    AF = mybir.ActivationFunctionType
    ALU = mybir.AluOpType
    AX = mybir.AxisListType

    nc = bacc.Bacc("TRN2", target_bir_lowering=False, debug=False, num_devices=1)

    enc_ext = nc.declare_dram_parameter("enc", [S, B, E], F32, isOutput=False)
    mask_ext = nc.declare_dram_parameter("mask", [S, B], U8, isOutput=False)
    dec_ext = nc.declare_dram_parameter("dec", [B, E], F32, isOutput=False)
    wenc_ext = nc.declare_dram_parameter("w_enc", [H, E], F32, isOutput=False)
    wdec_ext = nc.declare_dram_parameter("w_dec", [H, E], F32, isOutput=False)
    wout_ext = nc.declare_dram_parameter("w_out", [1, H], F32, isOutput=False)
    outw_ext = nc.declare_dram_parameter("out_w", [S, B], F32, isOutput=True)
    outr_ext = nc.declare_dram_parameter("out_resp", [B, E], F32, isOutput=True)

    with tile.TileContext(nc) as tc, ExitStack() as ctx:
        dram1 = ctx.enter_context(tc.tile_pool(name="dram1", bufs=1, space="DRAM"))
        dramc = ctx.enter_context(tc.tile_pool(name="dramc", bufs=16, space="DRAM"))
        persist = ctx.enter_context(tc.tile_pool(name="persist", bufs=1))
        sb_encT = ctx.enter_context(tc.tile_pool(name="sb_encT", bufs=24))
        sb_l = ctx.enter_context(tc.tile_pool(name="sb_l", bufs=2))
        sb_act = ctx.enter_context(tc.tile_pool(name="sb_act", bufs=10))
        sb_resp = ctx.enter_context(tc.tile_pool(name="sb_resp", bufs=3))
        sb_misc = ctx.enter_context(tc.tile_pool(name="sb_misc", bufs=1))
        ps_mm = ctx.enter_context(tc.tile_pool(name="ps_mm", bufs=2, space="PSUM"))
        ps_l = ctx.enter_context(tc.tile_pool(name="ps_l", bufs=2, space="PSUM"))
        ps_r = ctx.enter_context(tc.tile_pool(name="ps_r", bufs=2, space="PSUM"))
        ps_t = ctx.enter_context(tc.tile_pool(name="ps_t", bufs=2, space="PSUM"))

        # ---- phase 0: weight prep ----
        wenc_bf = dram1.tile([H, E], BF16, tag="wencbf")
        wdec_bf = dram1.tile([H, E], BF16, tag="wdecbf")
        nc.gpsimd.dma_start(out=wenc_bf[:, :], in_=wenc_ext[:, :])
        nc.gpsimd.dma_start(out=wdec_bf[:, :], in_=wdec_ext[:, :])
        wencT = persist.tile([128, ET, H], BF16, tag="wencT")
        wdecT = persist.tile([128, ET, H], BF16, tag="wdecT")
        for et in range(ET):
            nc.sync.dma_start_transpose(out=wencT[:, et, :], in_=wenc_bf[:, et * 128:(et + 1) * 128])
            nc.sync.dma_start_transpose(out=wdecT[:, et, :], in_=wdec_bf[:, et * 128:(et + 1) * 128])

        ident = persist.tile([128, 128], F32, tag="ident")
        make_identity(nc, ident[:, :])

        dec_sb = persist.tile([B, E], F32, tag="dec_sb")
        nc.sync.dma_start(out=dec_sb[:, :], in_=dec_ext[:, :])
        decT = persist.tile([128, ET, B], BF16, tag="decT")
        for dt_ in range(ET):
            tp = ps_t.tile([128, B], F32, tag="tp")
            nc.tensor.transpose(tp[:, :], dec_sb[:, dt_ * 128:(dt_ + 1) * 128], ident[0:B, 0:B])
            nc.vector.tensor_copy(decT[:, dt_, :], tp[:, :])

        wout_sb = persist.tile([1, H], F32, tag="wout_sb")
        nc.sync.dma_start(out=wout_sb[:, :], in_=wout_ext[:, :])
        woutT = persist.tile([128, HT, 1], BF16, tag="woutT")
        for ht in range(HT):
            tp = ps_t.tile([128, 1], F32, tag="tp")
            nc.tensor.transpose(tp[:, :], wout_sb[0:1, ht * 128:(ht + 1) * 128], ident[0:1, 0:1])
            nc.vector.tensor_copy(woutT[:, ht, :], tp[:, :])

        # proj_dec[h, b] in f32 (tanh bias)
        pd_sb = persist.tile([128, HT, B], F32, tag="pd_sb")
        for ht in range(HT):
            pp = ps_t.tile([128, B], F32, tag="tp")
            for dt_ in range(ET):
                nc.tensor.matmul(pp[:, :], wdecT[:, dt_, ht * 128:(ht + 1) * 128], decT[:, dt_, :],
                                 start=(dt_ == 0), stop=(dt_ == ET - 1))
            nc.vector.tensor_copy(pd_sb[:, ht, :], pp[:, :])

        # mask stays interleaved [s, b] on partition 0; rows extracted on the fly
        mask_sb = persist.tile([1, S, B], U8, tag="mask_sb")
        nc.sync.dma_start(out=mask_sb[:, :, :], in_=mask_ext[:, :])

        # logits / weights live interleaved in the [s, b] output staging row;
        # all softmax ops run in-place on stride-B views of it
        wflat = persist.tile([1, S, B], F32, tag="wflat")
        wTr = persist.tile([128, ST, B], F32R, tag="wTr")
        resp_sb = persist.tile([1, B, E], F32, tag="resp_sb")

        enc_bf_tiles = []

        def emit_softmax_and_resp(b, lrow):
            lb = lrow[:, :]
            mrow = sb_misc.tile([1, S], F32, tag="mrow")
            nc.vector.tensor_copy(mrow[:, :], mask_sb[:, :, b])
            # masked = logits*m + (m*2e9 - 2e9)
            nc.vector.tensor_mul(lb, lb, mrow[:, :])
            nc.vector.tensor_scalar(mrow[:, :], mrow[:, :], NEG_BIG, -NEG_BIG,
                                    ALU.mult, ALU.add)
            nc.vector.tensor_add(lb, lb, mrow[:, :])
            nmax = sb_misc.tile([1, 1], F32, tag="nmax")
            nc.vector.tensor_reduce(nmax[:, :], lb, AX.X, ALU.max, negate=True)
            ssum = sb_misc.tile([1, 1], F32, tag="ssum")
            nc.scalar.activation(lb, lb, AF.Exp, bias=nmax[:, :], scale=1.0,
                                 accum_out=ssum[:, :])
            rinv = sb_misc.tile([1, 1], F32, tag="rinv")
            nc.vector.reciprocal(rinv[:, :], ssum[:, :])
            nc.vector.tensor_scalar_mul(lb, lb, rinv[:, :])
            # stage the [s, b]-interleaved output row + transpose for the resp matmul
            nc.vector.tensor_copy(wflat[:, :, b], lb)
            for st in range(ST):
                tp = ps_t.tile([128, 1], F32, tag="tp")
                nc.tensor.transpose(tp[:, :], lrow[:, st * 128:(st + 1) * 128],
                                    ident[0:1, 0:1])
                nc.vector.tensor_copy(wTr[:, st, b:b + 1], tp[:, :])
            # attn_response[b, :] = sum_s weights[s] * enc[s, b, :]
            # f32r matmul: full-rate like bf16, ~16x better mantissa
            psR = [ps_r.tile([1, 512], F32, tag="psR", name=f"psR{b}_{eh}")
                   for eh in range(2)]
            for st in range(ST):
                rhs32 = sb_resp.tile([128, E], F32, tag="rhs32")
                srow = st * 128
                nc.sync.dma_start(out=rhs32[:, :], in_=enc_ext[srow:srow + 128, b, :])
                rhsr = sb_resp.tile([128, E], F32R, tag="rhsr")
                nc.vector.tensor_copy(rhsr[:, :], rhs32[:, :])
                for eh in range(2):
                    nc.tensor.matmul(psR[eh][:, :], wTr[:, st, b:b + 1],
                                     rhsr[:, eh * 512:(eh + 1) * 512],
                                     start=(st == 0), stop=(st == ST - 1))
            for eh in range(2):
                nc.vector.tensor_copy(resp_sb[:, b, eh * 512:(eh + 1) * 512], psR[eh][:, :])
            nc.sync.dma_start(out=outr_ext[b:b + 1, :], in_=resp_sb[:, b, :])

        # ---- main loop over (b, s-chunk) ----
        # enc bf16 scratch is e-tiled per batch column: [et, s, 128] so the
        # xbar transpose reads fully contiguous blocks (~4x the bandwidth of
        # the strided [s, e] layout).
        lrow = None
        ebf = None
        for c in range(B * NCHUNK_B):
            b, sc = divmod(c, NCHUNK_B)
            s0 = sc * SCH
            if sc == 0:
                lrow = sb_l.tile([1, S], F32, tag="lrow", name=f"lrow{b}")
                ebf = dramc.tile([ET, S, 128], BF16, tag="encbf", name=f"encbf{b}")
                src = enc_ext[:, b, :].rearrange("s (t e) -> t s e", e=128)
                # one cast-DMA per e-tile: strided-source SWDGE DMAs above
                # ~2k descriptors fail at runtime
                for et in range(ET):
                    nc.gpsimd.dma_start(out=ebf[et, :, :], in_=src[et, :, :])
            encT = []
            for et in range(ET):
                t = sb_encT.tile([128, SCH], BF16, tag="encT")
                nc.sync.dma_start_transpose(out=t[:, :], in_=ebf[et, s0:s0 + SCH, :])
                encT.append(t)
            lps = ps_l.tile([1, SCH], F32, tag="lps")
            pend = []
            for ht in range(HT):
                ps = ps_mm.tile([128, SCH], F32, tag="psmm")
                for et in range(ET):
                    nc.tensor.matmul(ps[:, :], wencT[:, et, ht * 128:(ht + 1) * 128], encT[et][:, :],
                                     start=(et == 0), stop=(et == ET - 1))
                act = sb_act.tile([128, SCH], BF16, tag="act")
                nc.scalar.activation(act[:, :], ps[:, :], AF.Tanh, bias=pd_sb[:, ht, b:b + 1],
                                     scale=1.0)
                pend.append((ht, act))
                # emit logits matmuls one h-tile behind so the PE never waits on ACT
                if len(pend) >= 2:
                    h0, a0 = pend.pop(0)
                    nc.tensor.matmul(lps[:, :], woutT[:, h0, :], a0[:, :],
                                     start=(h0 == 0), stop=(h0 == HT - 1))
            for h0, a0 in pend:
                nc.tensor.matmul(lps[:, :], woutT[:, h0, :], a0[:, :],
                                 start=(h0 == 0), stop=(h0 == HT - 1))
            nc.vector.tensor_copy(lrow[:, s0:s0 + SCH], lps[:, :])
            if sc == NCHUNK_B - 1:
                emit_softmax_and_resp(b, lrow)

        nc.sync.dma_start(out=outw_ext[:, :], in_=wflat[:, :, :])

    nc.compile()
    return nc


def _get_nc():
    if "nc" not in _CACHE:
        _CACHE["nc"] = _build()
    return _CACHE["nc"]


def kernel(enc_states, enc_mask, dec_state, w_enc, w_dec, w_out):
    from concourse.bass_utils import run_bass_kernel_spmd

    nc = _get_nc()
    enc_states = np.asarray(enc_states, dtype=np.float32)
    mask_u8 = np.asarray(enc_mask).astype(np.uint8)
    dec_state = np.asarray(dec_state, dtype=np.float32)
    w_enc = np.ascontiguousarray(np.asarray(w_enc, dtype=np.float32))
    w_dec = np.ascontiguousarray(np.asarray(w_dec, dtype=np.float32))
    w_out = np.ascontiguousarray(np.asarray(w_out, dtype=np.float32).reshape(1, H))

    in_maps = []
    for c in range(NCORES):
        bs = slice(c * B, (c + 1) * B)
        in_maps.append({
            "enc": np.ascontiguousarray(enc_states[:, bs, :]),
            "mask": np.ascontiguousarray(mask_u8[:, bs]),
            "dec": np.ascontiguousarray(dec_state[bs, :]),
            "w_enc": w_enc,
            "w_dec": w_dec,
            "w_out": w_out,
        })

    res = run_bass_kernel_spmd(nc, in_maps, core_ids=list(range(NCORES)))
    attn_weights = np.concatenate([res.results[c]["out_w"] for c in range(NCORES)], axis=1)
    attn_response = np.concatenate([res.results[c]["out_resp"] for c in range(NCORES)], axis=0)
    return attn_response.astype(np.float32), attn_weights.astype(np.float32)


# revision 24
# speedup vs baseline: 1.0368x; 1.0238x over previous
"""Bahdanau-style additive attention on 8 TRN2 NeuronCores.

Data-parallel over batch (32 -> 4 per core); zero collectives (softmax is
over the sequence axis, which stays local to each core).

Per-core pipeline (S=2048 local seq, B=4 local batch, E=H=1024):
  - cast enc f32->bf16 into DRAM scratch chunks (SWDGE cast-DMA)
  - DMA-transpose bf16 chunks into [e, r] tiles (xbar transpose)
  - proj_enc: PE matmul, stationary w_encT (bf16), out psum [h=128, r=512]
  - tanh(+proj_dec bias) on ACT -> bf16 activations
  - logits: PE matmul with w_out (M=1), accumulated over h-tiles
  - mask + softmax per batch column on DVE/ACT (free-dim reductions)
  - attn_response: PE matmul, stationary = weights column, moving = enc bf16
"""

import sys

import numpy as np

_BASS_PATH = "/opt/trn_rl_repo"
if _BASS_PATH not in sys.path:
    sys.path.insert(0, _BASS_PATH)

S, B_FULL, E, H = 2048, 32, 1024, 1024
NCORES = 8
B = B_FULL // NCORES            # 4 batch columns per core
NEG_BIG = 2.0e9
SCH = 512                       # seq rows per chunk
NCHUNK_B = S // SCH             # 4 chunks per batch column
ET, HT, ST = E // 128, H // 128, S // 128

_CACHE = {}


def _build():
    import concourse.tile as tile
    from concourse import bacc, mybir
    from concourse.masks import make_identity
    from contextlib import ExitStack

    F32 = mybir.dt.float32
    BF16 = mybir.dt.bfloat16
    U8 = mybir.dt.uint8
    F32R = mybir.dt.float32r
    AF = mybir.ActivationFunctionType
    ALU = mybir.AluOpType
    AX = mybir.AxisListType

    nc = bacc.Bacc("TRN2", target_bir_lowering=False, debug=False, num_devices=1)

    enc_ext = nc.declare_dram_parameter("enc", [S, B, E], F32, isOutput=False)
    mask_ext = nc.declare_dram_parameter("mask", [S, B], U8, isOutput=False)
    dec_ext = nc.declare_dram_parameter("dec", [B, E], F32, isOutput=False)
    wenc_ext = nc.declare_dram_parameter("w_enc", [H, E], F32, isOutput=False)
    wdec_ext = nc.declare_dram_parameter("w_dec", [H, E], F32, isOutput=False)
    wout_ext = nc.declare_dram_parameter("w_out", [1, H], F32, isOutput=False)
    outw_ext = nc.declare_dram_parameter("out_w", [S, B], F32, isOutput=True)
    outr_ext = nc.declare_dram_parameter("out_resp", [B, E], F32, isOutput=True)

    with tile.TileContext(nc) as tc, ExitStack() as ctx:
        dram1 = ctx.enter_context(tc.tile_pool(name="dram1", bufs=1, space="DRAM"))
        dramc = ctx.enter_context(tc.tile_pool(name="dramc", bufs=16, space="DRAM"))
        persist = ctx.enter_context(tc.tile_pool(name="persist", bufs=1))
        sb_encT = ctx.enter_context(tc.tile_pool(name="sb_encT", bufs=24))
        sb_l = ctx.enter_context(tc.tile_pool(name="sb_l", bufs=2))
        sb_act = ctx.enter_context(tc.tile_pool(name="sb_act", bufs=10))
        sb_resp = ctx.enter_context(tc.tile_pool(name="sb_resp", bufs=3))
        sb_misc = ctx.enter_context(tc.tile_pool(name="sb_misc", bufs=1))
        ps_mm = ctx.enter_context(tc.tile_pool(name="ps_mm", bufs=2, space="PSUM"))
        ps_l = ctx.enter_context(tc.tile_pool(name="ps_l", bufs=2, space="PSUM"))
        ps_r = ctx.enter_context(tc.tile_pool(name="ps_r", bufs=2, space="PSUM"))
        ps_t = ctx.enter_context(tc.tile_pool(name="ps_t", bufs=2, space="PSUM"))

        # ---- phase 0: weight prep ----
        wenc_bf = dram1.tile([H, E], BF16, tag="wencbf")
        wdec_bf = dram1.tile([H, E], BF16, tag="wdecbf")
        nc.gpsimd.dma_start(out=wenc_bf[:, :], in_=wenc_ext[:, :])
        nc.gpsimd.dma_start(out=wdec_bf[:, :], in_=wdec_ext[:, :])
        wencT = persist.tile([128, ET, H], BF16, tag="wencT")
        wdecT = persist.tile([128, ET, H], BF16, tag="wdecT")
        for et in range(ET):
            nc.sync.dma_start_transpose(out=wencT[:, et, :], in_=wenc_bf[:, et * 128:(et + 1) * 128])
            nc.sync.dma_start_transpose(out=wdecT[:, et, :], in_=wdec_bf[:, et * 128:(et + 1) * 128])

        ident = persist.tile([128, 128], F32, tag="ident")
        make_identity(nc, ident[:, :])

        dec_sb = persist.tile([B, E], F32, tag="dec_sb")
        nc.sync.dma_start(out=dec_sb[:, :], in_=dec_ext[:, :])
        decT = persist.tile([128, ET, B], BF16, tag="decT")
        for dt_ in range(ET):
            tp = ps_t.tile([128, B], F32, tag="tp")
            nc.tensor.transpose(tp[:, :], dec_sb[:, dt_ * 128:(dt_ + 1) * 128], ident[0:B, 0:B])
            nc.vector.tensor_copy(decT[:, dt_, :], tp[:, :])

        wout_sb = persist.tile([1, H], F32, tag="wout_sb")
        nc.sync.dma_start(out=wout_sb[:, :], in_=wout_ext[:, :])
        woutT = persist.tile([128, HT, 1], BF16, tag="woutT")
        for ht in range(HT):
            tp = ps_t.tile([128, 1], F32, tag="tp")
            nc.tensor.transpose(tp[:, :], wout_sb[0:1, ht * 128:(ht + 1) * 128], ident[0:1, 0:1])
            nc.vector.tensor_copy(woutT[:, ht, :], tp[:, :])

        # proj_dec[h, b] in f32 (tanh bias)
        pd_sb = persist.tile([128, HT, B], F32, tag="pd_sb")
        for ht in range(HT):
            pp = ps_t.tile([128, B], F32, tag="tp")
            for dt_ in range(ET):
                nc.tensor.matmul(pp[:, :], wdecT[:, dt_, ht * 128:(ht + 1) * 128], decT[:, dt_, :],
                                 start=(dt_ == 0), stop=(dt_ == ET - 1))
            nc.vector.tensor_copy(pd_sb[:, ht, :], pp[:, :])

        # mask stays interleaved [s, b] on partition 0; rows extracted on the fly
        mask_sb = persist.tile([1, S, B], U8, tag="mask_sb")
        nc.sync.dma_start(out=mask_sb[:, :, :], in_=mask_ext[:, :])

        # logits / weights live interleaved in the [s, b] output staging row;
        # all softmax ops run in-place on stride-B views of it
        wflat = persist.tile([1, S, B], F32, tag="wflat")
        wTr = persist.tile([128, ST, B], F32R, tag="wTr")
        resp_sb = persist.tile([1, B, E], F32, tag="resp_sb")

        enc_bf_tiles = []

        def emit_softmax_and_resp(b, lrow):
            lb = lrow[:, :]
            mrow = sb_misc.tile([1, S], F32, tag="mrow")
            nc.vector.tensor_copy(mrow[:, :], mask_sb[:, :, b])
            # masked = logits*m + (m*2e9 - 2e9)
            nc.vector.tensor_mul(lb, lb, mrow[:, :])
            nc.vector.tensor_scalar(mrow[:, :], mrow[:, :], NEG_BIG, -NEG_BIG,
                                    ALU.mult, ALU.add)
            nc.vector.tensor_add(lb, lb, mrow[:, :])
            nmax = sb_misc.tile([1, 1], F32, tag="nmax")
            nc.vector.tensor_reduce(nmax[:, :], lb, AX.X, ALU.max, negate=True)
            ssum = sb_misc.tile([1, 1], F32, tag="ssum")
            nc.scalar.activation(lb, lb, AF.Exp, bias=nmax[:, :], scale=1.0,
                                 accum_out=ssum[:, :])
            rinv = sb_misc.tile([1, 1], F32, tag="rinv")
            nc.vector.reciprocal(rinv[:, :], ssum[:, :])
            nc.vector.tensor_scalar_mul(lb, lb, rinv[:, :])
            # stage the [s, b]-interleaved output row + transpose for the resp matmul
            nc.vector.tensor_copy(wflat[:, :, b], lb)
            for st in range(ST):
                tp = ps_t.tile([128, 1], F32, tag="tp")
                nc.tensor.transpose(tp[:, :], lrow[:, st * 128:(st + 1) * 128],
                                    ident[0:1, 0:1])
                nc.vector.tensor_copy(wTr[:, st, b:b + 1], tp[:, :])
            # attn_response[b, :] = sum_s weights[s] * enc[s, b, :]
            # f32r matmul: full-rate like bf16, ~16x better mantissa
            psR = [ps_r.tile([1, 512], F32, tag="psR", name=f"psR{b}_{eh}")
                   for eh in range(2)]
            for st in range(ST):
                rhs32 = sb_resp.tile([128, E], F32, tag="rhs32")
                srow = st * 128
                nc.sync.dma_start(out=rhs32[:, :], in_=enc_ext[srow:srow + 128, b, :])
                rhsr = sb_resp.tile([128, E], F32R, tag="rhsr")
                nc.vector.tensor_copy(rhsr[:, :], rhs32[:, :])
                for eh in range(2):
                    nc.tensor.matmul(psR[eh][:, :], wTr[:, st, b:b + 1],
                                     rhsr[:, eh * 512:(eh + 1) * 512],
                                     start=(st == 0), stop=(st == ST - 1))
            for eh in range(2):
                nc.vector.tensor_copy(resp_sb[:, b, eh * 512:(eh + 1) * 512], psR[eh][:, :])
            nc.sync.dma_start(out=outr_ext[b:b + 1, :], in_=resp_sb[:, b, :])

        # ---- main loop over (b, s-chunk) ----
        # enc bf16 scratch is e-tiled per batch column: [et, s, 128] so the
        # xbar transpose reads fully contiguous blocks (~4x the bandwidth of
        # the strided [s, e] layout).
        lrow = None
        ebf = None
        for c in range(B * NCHUNK_B):
            b, sc = divmod(c, NCHUNK_B)
            s0 = sc * SCH
            if sc == 0:
                lrow = sb_l.tile([1, S], F32, tag="lrow", name=f"lrow{b}")
            ebf = dramc.tile([SCH, E], BF16, tag="encbf", name=f"encbf{c}")
            nc.gpsimd.dma_start(out=ebf[:, :], in_=enc_ext[s0:s0 + SCH, b, :])
            encT = []
            for et in range(ET):
                t = sb_encT.tile([128, SCH], BF16, tag="encT")
                nc.sync.dma_start_transpose(out=t[:, :], in_=ebf[:, et * 128:(et + 1) * 128])
                encT.append(t)
            lps = ps_l.tile([1, SCH], F32, tag="lps")
            pend = []
            for ht in range(HT):
                ps = ps_mm.tile([128, SCH], F32, tag="psmm")
                for et in range(ET):
                    nc.tensor.matmul(ps[:, :], wencT[:, et, ht * 128:(ht + 1) * 128], encT[et][:, :],
                                     start=(et == 0), stop=(et == ET - 1))
                act = sb_act.tile([128, SCH], BF16, tag="act")
                nc.scalar.activation(act[:, :], ps[:, :], AF.Tanh, bias=pd_sb[:, ht, b:b + 1],
                                     scale=1.0)
                pend.append((ht, act))
                # emit logits matmuls one h-tile behind so the PE never waits on ACT
                if len(pend) >= 2:
                    h0, a0 = pend.pop(0)
                    nc.tensor.matmul(lps[:, :], woutT[:, h0, :], a0[:, :],
                                     start=(h0 == 0), stop=(h0 == HT - 1))
            for h0, a0 in pend:
                nc.tensor.matmul(lps[:, :], woutT[:, h0, :], a0[:, :],
                                 start=(h0 == 0), stop=(h0 == HT - 1))
            nc.vector.tensor_copy(lrow[:, s0:s0 + SCH], lps[:, :])
            if sc == NCHUNK_B - 1:
                emit_softmax_and_resp(b, lrow)

        nc.sync.dma_start(out=outw_ext[:, :], in_=wflat[:, :, :])

    nc.compile()
    return nc


def _get_nc():
    if "nc" not in _CACHE:
        _CACHE["nc"] = _build()
    return _CACHE["nc"]


def kernel(enc_states, enc_mask, dec_state, w_enc, w_dec, w_out):
    from concourse.bass_utils import run_bass_kernel_spmd

    nc = _get_nc()
    enc_states = np.asarray(enc_states, dtype=np.float32)
    mask_u8 = np.asarray(enc_mask).astype(np.uint8)
    dec_state = np.asarray(dec_state, dtype=np.float32)
    w_enc = np.ascontiguousarray(np.asarray(w_enc, dtype=np.float32))
    w_dec = np.ascontiguousarray(np.asarray(w_dec, dtype=np.float32))
    w_out = np.ascontiguousarray(np.asarray(w_out, dtype=np.float32).reshape(1, H))

    in_maps = []
    for c in range(NCORES):
        bs = slice(c * B, (c + 1) * B)
        in_maps.append({
            "enc": np.ascontiguousarray(enc_states[:, bs, :]),
            "mask": np.ascontiguousarray(mask_u8[:, bs]),
            "dec": np.ascontiguousarray(dec_state[bs, :]),
            "w_enc": w_enc,
            "w_dec": w_dec,
            "w_out": w_out,
        })

    res = run_bass_kernel_spmd(nc, in_maps, core_ids=list(range(NCORES)))
    attn_weights = np.concatenate([res.results[c]["out_w"] for c in range(NCORES)], axis=1)
    attn_response = np.concatenate([res.results[c]["out_resp"] for c in range(NCORES)], axis=0)
    return attn_response.astype(np.float32), attn_weights.astype(np.float32)


# revision 26
# speedup vs baseline: 1.1522x; 1.1113x over previous
"""Bahdanau-style additive attention on 8 TRN2 NeuronCores.

Data-parallel over batch (32 -> 4 per core); zero collectives (softmax is
over the sequence axis, which stays local to each core).

Per-core pipeline (S=2048 local seq, B=4 local batch, E=H=1024):
  - cast enc f32->bf16 into DRAM scratch chunks (SWDGE cast-DMA)
  - DMA-transpose bf16 chunks into [e, r] tiles (xbar transpose)
  - proj_enc: PE matmul, stationary w_encT (bf16), out psum [h=128, r=512]
  - tanh(+proj_dec bias) on ACT -> bf16 activations
  - logits: PE matmul with w_out (M=1), accumulated over h-tiles
  - mask + softmax per batch column on DVE/ACT (free-dim reductions)
  - attn_response: PE matmul, stationary = weights column, moving = enc bf16
"""

import sys

import numpy as np

_BASS_PATH = "/opt/trn_rl_repo"
if _BASS_PATH not in sys.path:
    sys.path.insert(0, _BASS_PATH)

S, B_FULL, E, H = 2048, 32, 1024, 1024
NCORES = 8
B = B_FULL // NCORES            # 4 batch columns per core
NEG_BIG = 2.0e9
SCH = 512                       # seq rows per chunk
NCHUNK_B = S // SCH             # 4 chunks per batch column
ET, HT, ST = E // 128, H // 128, S // 128

_CACHE = {}


def _build():
    import concourse.tile as tile
    from concourse import bacc, mybir
    from concourse.masks import make_identity
    from contextlib import ExitStack

    F32 = mybir.dt.float32
    BF16 = mybir.dt.bfloat16
    U8 = mybir.dt.uint8
    F32R = mybir.dt.float32r
    AF = mybir.ActivationFunctionType
    ALU = mybir.AluOpType
    AX = mybir.AxisListType

    nc = bacc.Bacc("TRN2", target_bir_lowering=False, debug=False, num_devices=1)

    enc_ext = nc.declare_dram_parameter("enc", [S, B, E], F32, isOutput=False)
    mask_ext = nc.declare_dram_parameter("mask", [S, B], U8, isOutput=False)
    dec_ext = nc.declare_dram_parameter("dec", [B, E], F32, isOutput=False)
    wenc_ext = nc.declare_dram_parameter("w_enc", [H, E], F32, isOutput=False)
    wdec_ext = nc.declare_dram_parameter("w_dec", [H, E], F32, isOutput=False)
    wout_ext = nc.declare_dram_parameter("w_out", [1, H], F32, isOutput=False)
    outw_ext = nc.declare_dram_parameter("out_w", [S, B], F32, isOutput=True)
    outr_ext = nc.declare_dram_parameter("out_resp", [B, E], F32, isOutput=True)

    with tile.TileContext(nc) as tc, ExitStack() as ctx:
        dram1 = ctx.enter_context(tc.tile_pool(name="dram1", bufs=1, space="DRAM"))
        dramc = ctx.enter_context(tc.tile_pool(name="dramc", bufs=4, space="DRAM"))
        persist = ctx.enter_context(tc.tile_pool(name="persist", bufs=1))
        sb_encT = ctx.enter_context(tc.tile_pool(name="sb_encT", bufs=24))
        sb_l = ctx.enter_context(tc.tile_pool(name="sb_l", bufs=2))
        sb_act = ctx.enter_context(tc.tile_pool(name="sb_act", bufs=10))
        sb_resp = ctx.enter_context(tc.tile_pool(name="sb_resp", bufs=3))
        sb_misc = ctx.enter_context(tc.tile_pool(name="sb_misc", bufs=1))
        ps_mm = ctx.enter_context(tc.tile_pool(name="ps_mm", bufs=2, space="PSUM"))
        ps_l = ctx.enter_context(tc.tile_pool(name="ps_l", bufs=2, space="PSUM"))
        ps_r = ctx.enter_context(tc.tile_pool(name="ps_r", bufs=2, space="PSUM"))
        ps_t = ctx.enter_context(tc.tile_pool(name="ps_t", bufs=2, space="PSUM"))

        # ---- phase 0: weight prep ----
        wenc_bf = dram1.tile([H, E], BF16, tag="wencbf")
        wdec_bf = dram1.tile([H, E], BF16, tag="wdecbf")
        nc.gpsimd.dma_start(out=wenc_bf[:, :], in_=wenc_ext[:, :])
        nc.gpsimd.dma_start(out=wdec_bf[:, :], in_=wdec_ext[:, :])
        wencT = persist.tile([128, ET, H], BF16, tag="wencT")
        wdecT = persist.tile([128, ET, H], BF16, tag="wdecT")
        for et in range(ET):
            nc.sync.dma_start_transpose(out=wencT[:, et, :], in_=wenc_bf[:, et * 128:(et + 1) * 128])
            nc.sync.dma_start_transpose(out=wdecT[:, et, :], in_=wdec_bf[:, et * 128:(et + 1) * 128])

        ident = persist.tile([128, 128], F32, tag="ident")
        make_identity(nc, ident[:, :])

        dec_sb = persist.tile([B, E], F32, tag="dec_sb")
        nc.sync.dma_start(out=dec_sb[:, :], in_=dec_ext[:, :])
        decT = persist.tile([128, ET, B], BF16, tag="decT")
        for dt_ in range(ET):
            tp = ps_t.tile([128, B], F32, tag="tp")
            nc.tensor.transpose(tp[:, :], dec_sb[:, dt_ * 128:(dt_ + 1) * 128], ident[0:B, 0:B])
            nc.vector.tensor_copy(decT[:, dt_, :], tp[:, :])

        wout_sb = persist.tile([1, H], F32, tag="wout_sb")
        nc.sync.dma_start(out=wout_sb[:, :], in_=wout_ext[:, :])
        woutT = persist.tile([128, HT, 1], BF16, tag="woutT")
        for ht in range(HT):
            tp = ps_t.tile([128, 1], F32, tag="tp")
            nc.tensor.transpose(tp[:, :], wout_sb[0:1, ht * 128:(ht + 1) * 128], ident[0:1, 0:1])
            nc.vector.tensor_copy(woutT[:, ht, :], tp[:, :])

        # proj_dec[h, b] in f32 (tanh bias)
        pd_sb = persist.tile([128, HT, B], F32, tag="pd_sb")
        for ht in range(HT):
            pp = ps_t.tile([128, B], F32, tag="tp")
            for dt_ in range(ET):
                nc.tensor.matmul(pp[:, :], wdecT[:, dt_, ht * 128:(ht + 1) * 128], decT[:, dt_, :],
                                 start=(dt_ == 0), stop=(dt_ == ET - 1))
            nc.vector.tensor_copy(pd_sb[:, ht, :], pp[:, :])

        # mask stays interleaved [s, b] on partition 0; rows extracted on the fly
        mask_sb = persist.tile([1, S, B], U8, tag="mask_sb")
        nc.sync.dma_start(out=mask_sb[:, :, :], in_=mask_ext[:, :])

        # logits / weights live interleaved in the [s, b] output staging row;
        # all softmax ops run in-place on stride-B views of it
        wflat = persist.tile([1, S, B], F32, tag="wflat")
        wTr = persist.tile([128, ST, B], F32R, tag="wTr")
        resp_sb = persist.tile([1, B, E], F32, tag="resp_sb")

        enc_bf_tiles = []

        def emit_softmax_and_resp(b, lrow):
            lb = lrow[:, :]
            mrow = sb_misc.tile([1, S], F32, tag="mrow")
            nc.vector.tensor_copy(mrow[:, :], mask_sb[:, :, b])
            # masked = logits*m + (m*2e9 - 2e9)
            nc.vector.tensor_mul(lb, lb, mrow[:, :])
            nc.vector.tensor_scalar(mrow[:, :], mrow[:, :], NEG_BIG, -NEG_BIG,
                                    ALU.mult, ALU.add)
            nc.vector.tensor_add(lb, lb, mrow[:, :])
            nmax = sb_misc.tile([1, 1], F32, tag="nmax")
            nc.vector.tensor_reduce(nmax[:, :], lb, AX.X, ALU.max, negate=True)
            ssum = sb_misc.tile([1, 1], F32, tag="ssum")
            nc.scalar.activation(lb, lb, AF.Exp, bias=nmax[:, :], scale=1.0,
                                 accum_out=ssum[:, :])
            rinv = sb_misc.tile([1, 1], F32, tag="rinv")
            nc.vector.reciprocal(rinv[:, :], ssum[:, :])
            nc.vector.tensor_scalar_mul(lb, lb, rinv[:, :])
            # stage the [s, b]-interleaved output row + transpose for the resp matmul
            nc.vector.tensor_copy(wflat[:, :, b], lb)
            for st in range(ST):
                tp = ps_t.tile([128, 1], F32, tag="tp")
                nc.tensor.transpose(tp[:, :], lrow[:, st * 128:(st + 1) * 128],
                                    ident[0:1, 0:1])
                nc.vector.tensor_copy(wTr[:, st, b:b + 1], tp[:, :])
            # attn_response[b, :] = sum_s weights[s] * enc[s, b, :]
            # f32r matmul: full-rate like bf16, ~16x better mantissa
            psR = [ps_r.tile([1, 512], F32, tag="psR", name=f"psR{b}_{eh}")
                   for eh in range(2)]
            for st in range(ST):
                rhs32 = sb_resp.tile([128, E], F32, tag="rhs32")
                srow = st * 128
                nc.sync.dma_start(out=rhs32[:, :], in_=enc_ext[srow:srow + 128, b, :])
                rhsr = sb_resp.tile([128, E], F32R, tag="rhsr")
                nc.vector.tensor_copy(rhsr[:, :], rhs32[:, :])
                for eh in range(2):
                    nc.tensor.matmul(psR[eh][:, :], wTr[:, st, b:b + 1],
                                     rhsr[:, eh * 512:(eh + 1) * 512],
                                     start=(st == 0), stop=(st == ST - 1))
            for eh in range(2):
                nc.vector.tensor_copy(resp_sb[:, b, eh * 512:(eh + 1) * 512], psR[eh][:, :])
            nc.sync.dma_start(out=outr_ext[b:b + 1, :], in_=resp_sb[:, b, :])

        # ---- main loop over (b, s-chunk) ----
        # enc bf16 scratch is e-tiled per batch column: [et, s, 128] so the
        # xbar transpose reads fully contiguous blocks (~4x the bandwidth of
        # the strided [s, e] layout).
        lrow = None
        ebf = None
        for c in range(B * NCHUNK_B):
            b, sc = divmod(c, NCHUNK_B)
            s0 = sc * SCH
            if sc == 0:
                lrow = sb_l.tile([1, S], F32, tag="lrow", name=f"lrow{b}")
                ebf = dramc.tile([S, E], BF16, tag="encbf", name=f"encbf{b}")
                nc.gpsimd.dma_start(out=ebf[:, :], in_=enc_ext[:, b, :])
            encT = []
            for et in range(ET):
                t = sb_encT.tile([128, SCH], BF16, tag="encT")
                nc.sync.dma_start_transpose(
                    out=t[:, :], in_=ebf[s0:s0 + SCH, et * 128:(et + 1) * 128])
                encT.append(t)
            lps = ps_l.tile([1, SCH], F32, tag="lps")
            pend = []
            for ht in range(HT):
                ps = ps_mm.tile([128, SCH], F32, tag="psmm")
                for et in range(ET):
                    nc.tensor.matmul(ps[:, :], wencT[:, et, ht * 128:(ht + 1) * 128], encT[et][:, :],
                                     start=(et == 0), stop=(et == ET - 1))
                act = sb_act.tile([128, SCH], BF16, tag="act")
                nc.scalar.activation(act[:, :], ps[:, :], AF.Tanh, bias=pd_sb[:, ht, b:b + 1],
                                     scale=1.0)
                pend.append((ht, act))
                # emit logits matmuls one h-tile behind so the PE never waits on ACT
                if len(pend) >= 2:
                    h0, a0 = pend.pop(0)
                    nc.tensor.matmul(lps[:, :], woutT[:, h0, :], a0[:, :],
                                     start=(h0 == 0), stop=(h0 == HT - 1))
            for h0, a0 in pend:
                nc.tensor.matmul(lps[:, :], woutT[:, h0, :], a0[:, :],
                                 start=(h0 == 0), stop=(h0 == HT - 1))
            nc.vector.tensor_copy(lrow[:, s0:s0 + SCH], lps[:, :])
            if sc == NCHUNK_B - 1:
                emit_softmax_and_resp(b, lrow)

        nc.sync.dma_start(out=outw_ext[:, :], in_=wflat[:, :, :])

    nc.compile()
    return nc


def _get_nc():
    if "nc" not in _CACHE:
        _CACHE["nc"] = _build()
    return _CACHE["nc"]


def kernel(enc_states, enc_mask, dec_state, w_enc, w_dec, w_out):
    from concourse.bass_utils import run_bass_kernel_spmd

    nc = _get_nc()
    enc_states = np.asarray(enc_states, dtype=np.float32)
    mask_u8 = np.asarray(enc_mask).astype(np.uint8)
    dec_state = np.asarray(dec_state, dtype=np.float32)
    w_enc = np.ascontiguousarray(np.asarray(w_enc, dtype=np.float32))
    w_dec = np.ascontiguousarray(np.asarray(w_dec, dtype=np.float32))
    w_out = np.ascontiguousarray(np.asarray(w_out, dtype=np.float32).reshape(1, H))

    in_maps = []
    for c in range(NCORES):
        bs = slice(c * B, (c + 1) * B)
        in_maps.append({
            "enc": np.ascontiguousarray(enc_states[:, bs, :]),
            "mask": np.ascontiguousarray(mask_u8[:, bs]),
            "dec": np.ascontiguousarray(dec_state[bs, :]),
            "w_enc": w_enc,
            "w_dec": w_dec,
            "w_out": w_out,
        })

    res = run_bass_kernel_spmd(nc, in_maps, core_ids=list(range(NCORES)))
    attn_weights = np.concatenate([res.results[c]["out_w"] for c in range(NCORES)], axis=1)
    attn_response = np.concatenate([res.results[c]["out_resp"] for c in range(NCORES)], axis=0)
    return attn_response.astype(np.float32), attn_weights.astype(np.float32)
